# revision 21
# baseline (speedup 1.0000x reference)
"""Trainium2 Bass kernel for nn_LocalMQA (S=2048, D_MODEL=1024, H=16, D=64, WIN=128).

Sharding: sequence-parallel across 8 cores (256 output rows each) with a
128-row halo recomputed for k/v. No collectives; each core produces a
disjoint slice of the output.

Per-core pipeline (all fp16 matmuls, PSUM accumulates f32):
  - inputs DMA'd as large fully-contiguous per-partition transfers, split
    across the SP and ACT HWDGE rings; w2 queued behind w1 (needed last).
  - qkv: outer loop over K-chunks so PE consumes weight chunks as they
    stream in; all 9 PSUM accumulators open simultaneously.
  - attention per (it, h): maskbias copied into PSUM by DVE/ACT (not PE),
    score matmul accumulates on top; DVE rowmax(negate); ACT Exp with
    accum_out giving the softmax denominator Z for free; PE transpose;
    evac copy; AV matmul; batched DVE reciprocal of Z; scale-evac to o16.
  - per-it: o16 transposed to oT, outproj for that it runs immediately
    (overlaps the other it's attention), out DMA'd per (nt, it).
Host transposes/concats/casts the 8 outT slices into the final (2048, 1024).
"""
import numpy as np

import concourse.bacc as bacc
import concourse.mybir as mybir
import concourse.tile as tile
from concourse.bass_utils import run_bass_kernel_spmd

S = 2048
DM = 1024
H = 16
D = 64
WIN = 128
NC = 8
RPC = S // NC          # rows per core = 256
HALO = 128
XW = RPC + HALO        # per-core xT width = 384

F32 = mybir.dt.float32
F16 = mybir.dt.float16

_CACHED = {}


def _build(debug=False):
    nc = bacc.Bacc("TRN2", target_bir_lowering=False, debug=False, num_devices=NC)

    xT_d = nc.dram_tensor("xT", [128, 8 * XW], F16, kind="ExternalInput").ap()
    w1_d = nc.dram_tensor("w1T", [128, 8 * 1152], F16, kind="ExternalInput").ap()
    w2_d = nc.dram_tensor("w2T", [128, 8 * 1024], F16, kind="ExternalInput").ap()
    b1_d = nc.dram_tensor("b1", [128, 9], F32, kind="ExternalInput").ap()
    b2_d = nc.dram_tensor("b2", [128, 8], F32, kind="ExternalInput").ap()
    msk_d = nc.dram_tensor("mask", [128, 2, 512], F16, kind="ExternalInput").ap()
    id16_d = nc.dram_tensor("ident16", [128, 128], F16, kind="ExternalInput").ap()
    out_d = nc.dram_tensor("outT", [8, 2, 128, 128], F16, kind="ExternalOutput").ap()

    AF = mybir.ActivationFunctionType
    if debug:
        dbg = {
            "dbg_kv": nc.dram_tensor("dbg_kv", [128, XW], F16, kind="ExternalOutput").ap(),
            "dbg_q": nc.dram_tensor("dbg_q", [128, 8, 256], F16, kind="ExternalOutput").ap(),
            "dbg_v16t": nc.dram_tensor("dbg_v16t", [128, 3, 64], F16, kind="ExternalOutput").ap(),
            "dbg_attn": nc.dram_tensor("dbg_attn", [128, 2, 256], F16, kind="ExternalOutput").ap(),
            "dbg_z": nc.dram_tensor("dbg_z", [128, 16], F32, kind="ExternalOutput").ap(),
            "dbg_o16": nc.dram_tensor("dbg_o16", [128, 16, 64], F16, kind="ExternalOutput").ap(),
            "dbg_oT": nc.dram_tensor("dbg_oT", [128, 8, 128], F16, kind="ExternalOutput").ap(),
        }

    with tile.TileContext(nc) as tc:
      with (
        tc.tile_pool(name="w", bufs=1) as wp,      # weights + constants
        tc.tile_pool(name="act", bufs=1) as ap_,   # persistent activations
        tc.tile_pool(name="sm", bufs=8) as smp,    # small softmax tiles
        tc.tile_pool(name="att", bufs=6) as attp,
        tc.tile_pool(name="o16p", bufs=1) as o16p,
        tc.tile_pool(name="outp", bufs=4) as outp,
      ):
        xT = wp.tile([128, 8, XW], F16)
        w1 = wp.tile([128, 8, 1152], F16)
        w2 = wp.tile([128, 8, 1024], F16)
        b1 = wp.tile([128, 9], F32)
        b2 = wp.tile([128, 8], F32)
        msk = wp.tile([128, 2, 512], F16)
        id16 = wp.tile([128, 128], F16)

        # big inputs: contiguous transfers split across the two HWDGE rings.
        # sync(SP) ring: x-half0, w1-half0, w2-half0 ... out DMAs later
        # scalar(ACT) ring: x-half1, w1-half1, w2-half1
        xTf = xT[:].rearrange("p c n -> p (c n)")
        w1f = w1[:].rearrange("p c n -> p (c n)")
        w2f = w2[:].rearrange("p c n -> p (c n)")
        nc.sync.dma_start(xTf[:, 0:4 * XW], xT_d[:, 0:4 * XW])
        nc.scalar.dma_start(xTf[:, 4 * XW:8 * XW], xT_d[:, 4 * XW:8 * XW])
        nc.sync.dma_start(w1f[:, 0:4608], w1_d[:, 0:4608])
        nc.scalar.dma_start(w1f[:, 4608:9216], w1_d[:, 4608:9216])
        nc.sync.dma_start(w2f[:, 0:4096], w2_d[:, 0:4096])
        nc.scalar.dma_start(w2f[:, 4096:8192], w2_d[:, 4096:8192])
        # small constants on the gpsimd SWDGE ring
        nc.gpsimd.dma_start(id16[:], id16_d)
        nc.gpsimd.dma_start(b1[:], b1_d)
        nc.gpsimd.dma_start(b2[:], b2_d)
        nc.gpsimd.dma_start(msk[:], msk_d)

        # ---- qkv projection: consume w1 K-chunks as they arrive ----
        kv_sb = ap_.tile([128, XW], F16)       # k rows 0:64 (+ mirror 64:128)
        v16r = ap_.tile([128, 3, 128], F16)    # v (biased) at partitions 64:128
        v16t = ap_.tile([128, 3, 64], F16)     # v^T blocks [key, d]
        q_sb = ap_.tile([128, 8, 256], F16)    # q tiles, 2 heads per tile

        with (
            tc.tile_pool(name="ps_q", bufs=1, space="PSUM") as ps_q,
            tc.tile_pool(name="ps_kv", bufs=1, space="PSUM") as ps_kv,
        ):
            # one accumulation group per PSUM bank (start=True zeroes the
            # whole bank): kvp + 7 q tiles, then tile 7 reuses tile 0's bank.
            kvp = ps_kv.tile([128, XW], F32)
            qps = [ps_q.tile([128, 256], F32, tag=f"qp{t}", name=f"qp{t}")
                   for t in range(7)]

            def q_evac(t, qp):
                if t % 2 == 0:
                    nc.vector.tensor_scalar_add(q_sb[:, t, :], qp[:],
                                                b1[:, t + 1:t + 2])
                else:
                    nc.scalar.activation(q_sb[:, t, :], qp[:],
                                         AF.Identity, bias=b1[:, t + 1:t + 2],
                                         scale=1.0)

            for c in range(8):
                nc.tensor.matmul(kvp[:], w1[:, c, 0:128], xT[:, c, :],
                                 start=(c == 0), stop=(c == 7))
                for t in range(7):
                    nc.tensor.matmul(
                        qps[t][:], w1[:, c, 128 * (t + 1):128 * (t + 2)],
                        xT[:, c, HALO:XW],
                        start=(c == 0), stop=(c == 7))
            # k evac on ACT, v evac on DVE
            nc.scalar.activation(kv_sb[0:64, :], kvp[0:64, :],
                                 AF.Identity, bias=b1[0:64, 0:1], scale=1.0)
            nc.vector.tensor_scalar_add(
                v16r[64:128, :, :].rearrange("p b n -> p (b n)"), kvp[64:128, :],
                b1[64:128, 0:1])
            q_evac(0, qps[0])
            qp7 = ps_q.tile([128, 256], F32, tag="qp0", name="qp7")
            for c in range(8):
                nc.tensor.matmul(qp7[:], w1[:, c, 128 * 8:128 * 9],
                                 xT[:, c, HALO:XW],
                                 start=(c == 0), stop=(c == 7))
            for t in range(1, 7):
                q_evac(t, qps[t])
            q_evac(7, qp7)

        # v^T blocks + k mirror via gpsimd DMAs (off the weight rings)
        for b in range(3):
            nc.sync.dma_start(v16t[:, b, :], v16r[64:128, b, :], transpose=True)
        nc.gpsimd.dma_start(kv_sb[64:128, :], kv_sb[0:64, :])

        if debug:
            nc.gpsimd.dma_start(dbg["dbg_kv"], kv_sb[:])
            nc.gpsimd.dma_start(dbg["dbg_q"], q_sb[:])
            nc.gpsimd.dma_start(dbg["dbg_v16t"], v16t[:])

        with (
            tc.tile_pool(name="ps_s", bufs=2, space="PSUM") as ps_s,
            tc.tile_pool(name="ps_o", bufs=2, space="PSUM") as ps_o,
            tc.tile_pool(name="ps_t", bufs=2, space="PSUM") as ps_t,
            tc.tile_pool(name="ps_f", bufs=2, space="PSUM") as ps_f,
        ):
            for it in range(2):
                o16 = o16p.tile([128, 16, 64], F16, tag=f"o16_{it}")
                zt = o16p.tile([128, 16], F32, tag=f"z_{it}")
                rt = o16p.tile([128, 16], F32, tag=f"r_{it}")
                scb = None
                for h in range(16):
                    if h % 2 == 0:
                        scb = ps_s.tile([128, 512], F32, tag="sc", name="scb")
                    sc = scb[:, (h % 2) * 256:(h % 2) * 256 + 256]
                    # mask injection into PSUM by DVE/ACT (keeps PE free)
                    if h % 2 == 0:
                        # one inject fills mask for BOTH packed sc slots
                        # (start=True zeroes the whole bank, so it must be
                        # the bank's first write each rotation)
                        nc.tensor.matmul(scb[:], id16[:], msk[:, min(it, 1), :],
                                         start=True, stop=False)
                    nc.tensor.matmul(
                        sc[:],
                        q_sb[64 * (h % 2):64 * (h % 2) + 64, h // 2,
                             it * 128:it * 128 + 128],
                        kv_sb[64 * (h % 2):64 * (h % 2) + 64,
                              it * 128:it * 128 + 256],
                        start=False, stop=(h % 2 == 1), skip_group_check=True)
                    negm = smp.tile([128, 1], F32, tag="negm")
                    nc.vector.tensor_reduce(negm[:], sc[:],
                                            axis=mybir.AxisListType.X,
                                            op=mybir.AluOpType.max, negate=True)
                    attn = attp.tile([128, 256], F16, tag="attn")
                    nc.scalar.activation(attn[:], sc[:], AF.Exp,
                                         bias=negm[:], scale=1.0,
                                         accum_out=zt[:, h:h + 1])
                    if debug and it == 0 and h < 2:
                        nc.gpsimd.dma_start(dbg["dbg_attn"][:, h, :], attn[:])
                    ptt = ps_t.tile([128, 2, 128], F16, tag="tp")
                    for b in range(2):
                        nc.tensor.transpose(ptt[:, b, :],
                                            attn[:, b * 128:b * 128 + 128],
                                            id16[:])
                    attnT = attp.tile([128, 2, 128], F16, tag="attnT")
                    if h % 2 == 0:
                        nc.scalar.activation(
                            attnT[:].rearrange("p a b -> p (a b)"),
                            ptt[:].rearrange("p a b -> p (a b)"), AF.Copy)
                    else:
                        nc.vector.tensor_copy(
                            attnT[:].rearrange("p a b -> p (a b)"),
                            ptt[:].rearrange("p a b -> p (a b)"))
                    po = ps_o.tile([128, 64], F32, tag="po")
                    for b in range(2):
                        nc.tensor.matmul(po[:], attnT[:, b, :],
                                         v16t[:, it + b, :],
                                         start=(b == 0), stop=(b == 1))
                    nc.vector.reciprocal(rt[:, h:h + 1], zt[:, h:h + 1])
                    if h % 2 == 0:
                        nc.vector.tensor_scalar_mul(o16[:, h, :], po[:],
                                                    rt[:, h:h + 1])
                    else:
                        nc.scalar.activation(o16[:, h, :], po[:],
                                             AF.Copy, scale=rt[:, h:h + 1])

                if debug and it == 0:
                    nc.gpsimd.dma_start(dbg["dbg_z"], zt[:])
                    nc.gpsimd.dma_start(dbg["dbg_o16"], o16[:])
                # oT for this it: transpose o16 [128, 1024] -> 8 chunks [128,128]
                oT = o16p.tile([128, 8, 128], F16, tag=f"oT_{it}")
                for c in range(8):
                    pt = ps_t.tile([128, 2, 128], F16, tag="tp")
                    nc.tensor.transpose(
                        pt[:, 0, :],
                        o16[:, 2 * c:2 * c + 2, :].rearrange("p a b -> p (a b)"),
                        id16[:])
                    if c % 2 == 0:
                        nc.scalar.activation(oT[:, c, :], pt[:, 0, :], AF.Copy)
                    else:
                        nc.vector.tensor_copy(oT[:, c, :], pt[:, 0, :])

                if debug and it == 0:
                    nc.gpsimd.dma_start(dbg["dbg_oT"], oT[:])
                # outproj for this it (overlaps the other it's attention)
                for nt in range(8):
                    pf = ps_f.tile([128, 128], F32, tag="pf")
                    for c in range(8):
                        nc.tensor.matmul(pf[:], w2[:, c, 128 * nt:128 * (nt + 1)],
                                         oT[:, c, :],
                                         start=(c == 0), stop=(c == 7))
                    ot = outp.tile([128, 128], F16, tag="ot")
                    if nt % 2 == 0:
                        nc.scalar.activation(ot[:], pf[:], AF.Identity,
                                             bias=b2[:, nt:nt + 1], scale=1.0)
                    else:
                        nc.vector.tensor_scalar_add(ot[:], pf[:],
                                                    b2[:, nt:nt + 1])
                    nc.sync.dma_start(out_d[nt, it], ot[:])

    nc.compile()
    return nc


def _prep_inputs(x, Wqkv, bqkv, Wout, bout):
    x = np.asarray(x, dtype=np.float32)
    Wqkv = np.asarray(Wqkv, dtype=np.float32)
    bqkv = np.asarray(bqkv, dtype=np.float32)
    Wout = np.asarray(Wout, dtype=np.float32)
    bout = np.asarray(bout, dtype=np.float32)

    sq = np.sqrt(np.float32(D))
    W1 = Wqkv.copy()
    b1 = bqkv.copy()
    W1[2 * D:] *= sq
    b1[2 * D:] *= sq
    w1T = np.ascontiguousarray(
        W1.T.reshape(8, 128, 1152).transpose(1, 0, 2).reshape(128, 8 * 1152)
    ).astype(np.float16)
    b1t = np.ascontiguousarray(b1.reshape(9, 128).T)          # [128, 9]
    w2T = np.ascontiguousarray(
        Wout.T.reshape(8, 128, 1024).transpose(1, 0, 2).reshape(128, 8 * 1024)
    ).astype(np.float16)
    b2t = np.ascontiguousarray(bout.reshape(8, 128).T)        # [128, 8]

    pi = np.arange(128)[:, None]
    fj = np.arange(256)[None, :]
    std = np.where((fj > pi) & (fj <= pi + 128), 0.0, -60000.0).astype(np.float16)
    edge = np.where((fj > pi) & (fj <= pi + 128) & (fj >= 128), 0.0,
                    -60000.0).astype(np.float16)
    ident = np.eye(128, dtype=np.float16)

    in_maps = []
    for c in range(NC):
        r0 = c * RPC
        xs = np.zeros((XW, DM), np.float32)
        lo = max(0, r0 - HALO)
        xs[HALO - (r0 - lo):HALO + RPC] = x[lo:r0 + RPC]
        xTc = np.ascontiguousarray(
            xs.T.reshape(8, 128, XW).transpose(1, 0, 2).reshape(128, 8 * XW)
        ).astype(np.float16)
        m0 = edge if c == 0 else std
        mc = np.ascontiguousarray(
            np.stack([np.concatenate([m0, m0], 1),
                      np.concatenate([std, std], 1)], axis=1))  # [128, 2, 512]
        in_maps.append({
            "xT": xTc, "w1T": w1T, "b1": b1t, "w2T": w2T, "b2": b2t,
            "mask": mc, "ident16": ident,
        })
    return in_maps


def kernel(x, Wqkv, bqkv, Wout, bout):
    if "nc" not in _CACHED:
        _CACHED["nc"] = _build()
    nc = _CACHED["nc"]
    in_maps = _prep_inputs(x, Wqkv, bqkv, Wout, bout)
    res = run_bass_kernel_spmd(nc, in_maps, list(range(NC)))
    out = np.empty((S, DM), np.float32)
    for c in range(NC):
        outT = res.results[c]["outT"]          # [8, 2, 128, 128]
        full = outT.transpose(0, 2, 1, 3).reshape(DM, RPC)
        out[c * RPC:(c + 1) * RPC] = full.T.astype(np.float32)
    return out


if __name__ == "__main__":
    rng = np.random.default_rng(0)
    ins = {
        "x": rng.standard_normal((S, DM)).astype(np.float32),
        "Wqkv": (rng.standard_normal((1152, DM)) / 32).astype(np.float32),
        "bqkv": (rng.standard_normal((1152,)) * 0.01).astype(np.float32),
        "Wout": (rng.standard_normal((DM, DM)) / 32).astype(np.float32),
        "bout": (rng.standard_normal((DM,)) * 0.01).astype(np.float32),
    }
    out = kernel(**ins)
    print("kernel ran, out shape", out.shape)


# revision 26
# speedup vs baseline: 1.0389x; 1.0389x over previous
"""Trainium2 Bass kernel for nn_LocalMQA (S=2048, D_MODEL=1024, H=16, D=64, WIN=128).

Sharding: sequence-parallel across 8 cores (256 output rows each) with a
128-row halo recomputed for k/v. No collectives; each core produces a
disjoint slice of the output.

Per-core pipeline (all fp16 matmuls, PSUM accumulates f32):
  - inputs DMA'd as large fully-contiguous per-partition transfers, split
    across the SP and ACT HWDGE rings; w2 queued behind w1 (needed last).
  - qkv: outer loop over K-chunks so PE consumes weight chunks as they
    stream in; all 9 PSUM accumulators open simultaneously.
  - attention per (it, h): maskbias copied into PSUM by DVE/ACT (not PE),
    score matmul accumulates on top; DVE rowmax(negate); ACT Exp with
    accum_out giving the softmax denominator Z for free; PE transpose;
    evac copy; AV matmul; batched DVE reciprocal of Z; scale-evac to o16.
  - per-it: o16 transposed to oT, outproj for that it runs immediately
    (overlaps the other it's attention), out DMA'd per (nt, it).
Host transposes/concats/casts the 8 outT slices into the final (2048, 1024).
"""
import numpy as np

import concourse.bacc as bacc
import concourse.mybir as mybir
import concourse.tile as tile
from concourse.bass_utils import run_bass_kernel_spmd

S = 2048
DM = 1024
H = 16
D = 64
WIN = 128
NC = 8
RPC = S // NC          # rows per core = 256
HALO = 128
XW = RPC + HALO        # per-core xT width = 384

F32 = mybir.dt.float32
F16 = mybir.dt.float16

_CACHED = {}


def _build(debug=False):
    nc = bacc.Bacc("TRN2", target_bir_lowering=False, debug=False, num_devices=NC)

    xT_d = nc.dram_tensor("xT", [128, 8 * XW], F16, kind="ExternalInput").ap()
    # w1 tile-major: [p, out-tile(9: kv,q0..q7), K-chunk(8), 128]
    w1_d = nc.dram_tensor("w1T", [128, 9 * 8 * 128], F16, kind="ExternalInput").ap()
    w2_d = nc.dram_tensor("w2T", [128, 8 * 1024], F16, kind="ExternalInput").ap()
    b1_d = nc.dram_tensor("b1", [128, 9], F32, kind="ExternalInput").ap()
    b2_d = nc.dram_tensor("b2", [128, 8], F32, kind="ExternalInput").ap()
    msk_d = nc.dram_tensor("mask", [128, 2, 512], F16, kind="ExternalInput").ap()
    id16_d = nc.dram_tensor("ident16", [128, 128], F16, kind="ExternalInput").ap()
    out_d = nc.dram_tensor("outT", [8, 2, 128, 128], F16, kind="ExternalOutput").ap()

    AF = mybir.ActivationFunctionType
    if debug:
        dbg = {
            "dbg_kv": nc.dram_tensor("dbg_kv", [128, XW], F16, kind="ExternalOutput").ap(),
            "dbg_q": nc.dram_tensor("dbg_q", [128, 8, 256], F16, kind="ExternalOutput").ap(),
            "dbg_v16t": nc.dram_tensor("dbg_v16t", [128, 3, 64], F16, kind="ExternalOutput").ap(),
            "dbg_attn": nc.dram_tensor("dbg_attn", [128, 2, 256], F16, kind="ExternalOutput").ap(),
            "dbg_z": nc.dram_tensor("dbg_z", [128, 16], F32, kind="ExternalOutput").ap(),
            "dbg_o16": nc.dram_tensor("dbg_o16", [128, 16, 64], F16, kind="ExternalOutput").ap(),
            "dbg_oT": nc.dram_tensor("dbg_oT", [128, 8, 128], F16, kind="ExternalOutput").ap(),
        }

    with tile.TileContext(nc) as tc:
      with (
        tc.tile_pool(name="w", bufs=1) as wp,      # weights + constants
        tc.tile_pool(name="act", bufs=1) as ap_,   # persistent activations
        tc.tile_pool(name="sm", bufs=8) as smp,    # small softmax tiles
        tc.tile_pool(name="att", bufs=6) as attp,
        tc.tile_pool(name="o16p", bufs=1) as o16p,
        tc.tile_pool(name="outp", bufs=4) as outp,
      ):
        xT = wp.tile([128, 8, XW], F16)
        w1 = wp.tile([128, 9, 8, 128], F16)
        w2 = wp.tile([128, 8, 1024], F16)
        b1 = wp.tile([128, 9], F32)
        b2 = wp.tile([128, 8], F32)
        msk = wp.tile([128, 2, 512], F16)
        id16 = wp.tile([128, 128], F16)

        # big inputs: contiguous transfers split across the two HWDGE rings.
        # sync(SP) ring: x-half0, w1-half0, w2-half0 ... out DMAs later
        # scalar(ACT) ring: x-half1, w1-half1, w2-half1
        xTf = xT[:].rearrange("p c n -> p (c n)")
        w1f = w1[:].rearrange("p t c n -> p (t c n)")
        w2f = w2[:].rearrange("p c n -> p (c n)")
        nc.sync.dma_start(xTf[:, 0:4 * XW], xT_d[:, 0:4 * XW])
        nc.scalar.dma_start(xTf[:, 4 * XW:8 * XW], xT_d[:, 4 * XW:8 * XW])
        # w1 per out-tile (kv tile first), alternating rings so each q tile
        # lands as the previous one's matmuls run
        for t in range(9):
            eng = nc.sync if t % 2 == 0 else nc.scalar
            eng.dma_start(w1f[:, t * 1024:(t + 1) * 1024],
                          w1_d[:, t * 1024:(t + 1) * 1024])
        nc.sync.dma_start(w2f[:, 0:4096], w2_d[:, 0:4096])
        nc.scalar.dma_start(w2f[:, 4096:8192], w2_d[:, 4096:8192])
        # small constants on the gpsimd SWDGE ring
        nc.gpsimd.dma_start(id16[:], id16_d)
        nc.gpsimd.dma_start(b1[:], b1_d)
        nc.gpsimd.dma_start(b2[:], b2_d)
        nc.gpsimd.dma_start(msk[:], msk_d)

        # ---- qkv projection: consume w1 K-chunks as they arrive ----
        kv_sb = ap_.tile([128, XW], F16)       # k rows 0:64 (+ mirror 64:128)
        v16r = ap_.tile([128, 3, 128], F16)    # v (biased) at partitions 64:128
        v16t = ap_.tile([128, 3, 64], F16)     # v^T blocks [key, d]
        q_sb = ap_.tile([128, 8, 256], F16)    # q tiles, 2 heads per tile

        with (
            tc.tile_pool(name="ps_q", bufs=3, space="PSUM") as ps_q,
            tc.tile_pool(name="ps_kv", bufs=1, space="PSUM") as ps_kv,
        ):
            # t-outer: kv tile first (unblocks attention), each q tile's
            # matmuls start as soon as its weight slice lands
            kvp = ps_kv.tile([128, XW], F32)
            for c in range(8):
                nc.tensor.matmul(kvp[:], w1[:, 0, c, :], xT[:, c, :],
                                 start=(c == 0), stop=(c == 7))
            # k evac on ACT, v evac on DVE
            nc.scalar.activation(kv_sb[0:64, :], kvp[0:64, :],
                                 AF.Identity, bias=b1[0:64, 0:1], scale=1.0)
            nc.vector.tensor_scalar_add(
                v16r[64:128, :, :].rearrange("p b n -> p (b n)"), kvp[64:128, :],
                b1[64:128, 0:1])
            for t in range(8):
                qp = ps_q.tile([128, 256], F32, tag="qp")
                for c in range(8):
                    nc.tensor.matmul(qp[:], w1[:, t + 1, c, :],
                                     xT[:, c, HALO:XW],
                                     start=(c == 0), stop=(c == 7))
                if t % 2 == 0:
                    nc.vector.tensor_scalar_add(q_sb[:, t, :], qp[:],
                                                b1[:, t + 1:t + 2])
                else:
                    nc.scalar.activation(q_sb[:, t, :], qp[:],
                                         AF.Identity, bias=b1[:, t + 1:t + 2],
                                         scale=1.0)

        # v^T blocks + k mirror via gpsimd DMAs (off the weight rings)
        for b in range(3):
            nc.sync.dma_start(v16t[:, b, :], v16r[64:128, b, :], transpose=True)
        nc.gpsimd.dma_start(kv_sb[64:128, :], kv_sb[0:64, :])

        if debug:
            nc.gpsimd.dma_start(dbg["dbg_kv"], kv_sb[:])
            nc.gpsimd.dma_start(dbg["dbg_q"], q_sb[:])
            nc.gpsimd.dma_start(dbg["dbg_v16t"], v16t[:])

        with (
            tc.tile_pool(name="ps_s", bufs=2, space="PSUM") as ps_s,
            tc.tile_pool(name="ps_o", bufs=2, space="PSUM") as ps_o,
            tc.tile_pool(name="ps_t", bufs=2, space="PSUM") as ps_t,
            tc.tile_pool(name="ps_f", bufs=2, space="PSUM") as ps_f,
        ):
            for it in range(2):
                o16 = o16p.tile([128, 16, 64], F16, tag=f"o16_{it}")
                zt = o16p.tile([128, 16], F32, tag=f"z_{it}")
                rt = o16p.tile([128, 16], F32, tag=f"r_{it}")
                scb = None
                for h in range(16):
                    if h % 2 == 0:
                        scb = ps_s.tile([128, 512], F32, tag="sc", name="scb")
                    sc = scb[:, (h % 2) * 256:(h % 2) * 256 + 256]
                    # mask injection into PSUM by DVE/ACT (keeps PE free)
                    if h % 2 == 0:
                        # one inject fills mask for BOTH packed sc slots
                        # (start=True zeroes the whole bank, so it must be
                        # the bank's first write each rotation)
                        nc.tensor.matmul(scb[:], id16[:], msk[:, min(it, 1), :],
                                         start=True, stop=False)
                    nc.tensor.matmul(
                        sc[:],
                        q_sb[64 * (h % 2):64 * (h % 2) + 64, h // 2,
                             it * 128:it * 128 + 128],
                        kv_sb[64 * (h % 2):64 * (h % 2) + 64,
                              it * 128:it * 128 + 256],
                        start=False, stop=(h % 2 == 1), skip_group_check=True)
                    negm = smp.tile([128, 1], F32, tag="negm")
                    nc.vector.tensor_reduce(negm[:], sc[:],
                                            axis=mybir.AxisListType.X,
                                            op=mybir.AluOpType.max, negate=True)
                    attn = attp.tile([128, 256], F16, tag="attn")
                    nc.scalar.activation(attn[:], sc[:], AF.Exp,
                                         bias=negm[:], scale=1.0,
                                         accum_out=zt[:, h:h + 1])
                    if debug and it == 0 and h < 2:
                        nc.gpsimd.dma_start(dbg["dbg_attn"][:, h, :], attn[:])
                    ptt = ps_t.tile([128, 2, 128], F16, tag="tp")
                    for b in range(2):
                        nc.tensor.transpose(ptt[:, b, :],
                                            attn[:, b * 128:b * 128 + 128],
                                            id16[:])
                    attnT = attp.tile([128, 2, 128], F16, tag="attnT")
                    if h % 2 == 0:
                        nc.scalar.activation(
                            attnT[:].rearrange("p a b -> p (a b)"),
                            ptt[:].rearrange("p a b -> p (a b)"), AF.Copy)
                    else:
                        nc.vector.tensor_copy(
                            attnT[:].rearrange("p a b -> p (a b)"),
                            ptt[:].rearrange("p a b -> p (a b)"))
                    po = ps_o.tile([128, 64], F32, tag="po")
                    for b in range(2):
                        nc.tensor.matmul(po[:], attnT[:, b, :],
                                         v16t[:, it + b, :],
                                         start=(b == 0), stop=(b == 1))
                    nc.vector.reciprocal(rt[:, h:h + 1], zt[:, h:h + 1])
                    if h % 2 == 0:
                        nc.vector.tensor_scalar_mul(o16[:, h, :], po[:],
                                                    rt[:, h:h + 1])
                    else:
                        nc.scalar.activation(o16[:, h, :], po[:],
                                             AF.Copy, scale=rt[:, h:h + 1])

                if debug and it == 0:
                    nc.gpsimd.dma_start(dbg["dbg_z"], zt[:])
                    nc.gpsimd.dma_start(dbg["dbg_o16"], o16[:])
                # oT for this it: transpose o16 [128, 1024] -> 8 chunks [128,128]
                oT = o16p.tile([128, 8, 128], F16, tag=f"oT_{it}")
                for c in range(8):
                    pt = ps_t.tile([128, 2, 128], F16, tag="tp")
                    nc.tensor.transpose(
                        pt[:, 0, :],
                        o16[:, 2 * c:2 * c + 2, :].rearrange("p a b -> p (a b)"),
                        id16[:])
                    if c % 2 == 0:
                        nc.scalar.activation(oT[:, c, :], pt[:, 0, :], AF.Copy)
                    else:
                        nc.vector.tensor_copy(oT[:, c, :], pt[:, 0, :])

                if debug and it == 0:
                    nc.gpsimd.dma_start(dbg["dbg_oT"], oT[:])
                # outproj for this it (overlaps the other it's attention)
                for nt in range(8):
                    pf = ps_f.tile([128, 128], F32, tag="pf")
                    for c in range(8):
                        nc.tensor.matmul(pf[:], w2[:, c, 128 * nt:128 * (nt + 1)],
                                         oT[:, c, :],
                                         start=(c == 0), stop=(c == 7))
                    ot = outp.tile([128, 128], F16, tag="ot")
                    if nt % 2 == 0:
                        nc.scalar.activation(ot[:], pf[:], AF.Identity,
                                             bias=b2[:, nt:nt + 1], scale=1.0)
                    else:
                        nc.vector.tensor_scalar_add(ot[:], pf[:],
                                                    b2[:, nt:nt + 1])
                    nc.sync.dma_start(out_d[nt, it], ot[:])

    nc.compile()
    return nc


def _prep_inputs(x, Wqkv, bqkv, Wout, bout):
    x = np.asarray(x, dtype=np.float32)
    Wqkv = np.asarray(Wqkv, dtype=np.float32)
    bqkv = np.asarray(bqkv, dtype=np.float32)
    Wout = np.asarray(Wout, dtype=np.float32)
    bout = np.asarray(bout, dtype=np.float32)

    sq = np.sqrt(np.float32(D))
    W1 = Wqkv.copy()
    b1 = bqkv.copy()
    W1[2 * D:] *= sq
    b1[2 * D:] *= sq
    # [p, t, c, o] = W1[128t+o, 128c+p]
    w1T = np.ascontiguousarray(
        W1.reshape(9, 128, 8, 128).transpose(3, 0, 2, 1).reshape(128, 9 * 8 * 128)
    ).astype(np.float16)
    b1t = np.ascontiguousarray(b1.reshape(9, 128).T)          # [128, 9]
    w2T = np.ascontiguousarray(
        Wout.T.reshape(8, 128, 1024).transpose(1, 0, 2).reshape(128, 8 * 1024)
    ).astype(np.float16)
    b2t = np.ascontiguousarray(bout.reshape(8, 128).T)        # [128, 8]

    pi = np.arange(128)[:, None]
    fj = np.arange(256)[None, :]
    std = np.where((fj > pi) & (fj <= pi + 128), 0.0, -60000.0).astype(np.float16)
    edge = np.where((fj > pi) & (fj <= pi + 128) & (fj >= 128), 0.0,
                    -60000.0).astype(np.float16)
    ident = np.eye(128, dtype=np.float16)

    in_maps = []
    for c in range(NC):
        r0 = c * RPC
        xs = np.zeros((XW, DM), np.float32)
        lo = max(0, r0 - HALO)
        xs[HALO - (r0 - lo):HALO + RPC] = x[lo:r0 + RPC]
        xTc = np.ascontiguousarray(
            xs.T.reshape(8, 128, XW).transpose(1, 0, 2).reshape(128, 8 * XW)
        ).astype(np.float16)
        m0 = edge if c == 0 else std
        mc = np.ascontiguousarray(
            np.stack([np.concatenate([m0, m0], 1),
                      np.concatenate([std, std], 1)], axis=1))  # [128, 2, 512]
        in_maps.append({
            "xT": xTc, "w1T": w1T, "b1": b1t, "w2T": w2T, "b2": b2t,
            "mask": mc, "ident16": ident,
        })
    return in_maps


def kernel(x, Wqkv, bqkv, Wout, bout):
    if "nc" not in _CACHED:
        _CACHED["nc"] = _build()
    nc = _CACHED["nc"]
    in_maps = _prep_inputs(x, Wqkv, bqkv, Wout, bout)
    res = run_bass_kernel_spmd(nc, in_maps, list(range(NC)))
    out = np.empty((S, DM), np.float32)
    for c in range(NC):
        outT = res.results[c]["outT"]          # [8, 2, 128, 128]
        full = outT.transpose(0, 2, 1, 3).reshape(DM, RPC)
        out[c * RPC:(c + 1) * RPC] = full.T.astype(np.float32)
    return out


if __name__ == "__main__":
    rng = np.random.default_rng(0)
    ins = {
        "x": rng.standard_normal((S, DM)).astype(np.float32),
        "Wqkv": (rng.standard_normal((1152, DM)) / 32).astype(np.float32),
        "bqkv": (rng.standard_normal((1152,)) * 0.01).astype(np.float32),
        "Wout": (rng.standard_normal((DM, DM)) / 32).astype(np.float32),
        "bout": (rng.standard_normal((DM,)) * 0.01).astype(np.float32),
    }
    out = kernel(**ins)
    print("kernel ran, out shape", out.shape)
